# revision 24
# baseline (speedup 1.0000x reference)
"""DiagonalLinear (Toeplitz linear) Trainium2 kernel — Karatsuba v14.

y[b,s,o] = sum_i x[b,s,i] * W[o,i] + bias[o],  W[o,i] = vals[(i-o) mod 8191]
x: [4, 2048, 4096] f32 -> bf16 operands, f32 PSUM, f16 output (cast on host).

Data-parallel over 8 cores (1024 rows each). Within a core, the 4096x4096
Toeplitz matmul is decomposed with 3 levels of Karatsuba on the 2x2 block
structure W = [[A,B],[C,A]] (diagonal blocks of a Toeplitz matrix repeat):
  y_left  = A x0 + B x1 = P0 + P2,   P0 = A(x0+x1), P2 = (B-A) x1
  y_right = C x0 + A x1 = P0 + P3,   P3 = (C-A) x0
Recursing 3x gives 27 leaf products of [512x512] Toeplitz blocks = 108
N=512 matmuls per 128-row tile instead of 256 (42% of the MACs). Each
leaf block is a free-dim slice of a [128 x 896] periodic table built
host-side from +/- combinations of shifted `vals`.

Engine split (measured rates): PE runs the 864 matmuls gap-free at the
215.8ns N=512 issue floor; Scalar drains all 27 leaf PSUMs per row-tile
to f16 SBUF (~690ns each); DVE does every add in 16-bit SBUF (2x mode,
~425ns per 512-col add) — input combo tree, nl/nr, L1, root+bias.
GpSimd is unused for compute: it shares SBUF ports with DVE and running
them concurrently stalls DVE ~6x. The next row-tile's combos are emitted
on DVE before the current tile's tail adds, and each tile's root adds
are deferred two tiles (DVE enters the first tiles ~7us behind from the
mc0 combo bootstrap), so the PE stream never breaks at a row-tile
boundary. Startup orders table/xt DMAs by first-need on both HWDGE
queues with later tables as 4-table batched transfers (startup is
supply-bound at ~0.43MB/us from t~8us; reordering cannot beat it). The
last row-tile runs group 0 first and finishes group 3 in column halves
to shorten the tail. HW: ~224us on 8 cores (baseline 464us), rel err
~5.8e-3 (gate 2e-2).
"""

import numpy as np
import ml_dtypes

import bass_rust
import concourse.bass as bass
import concourse.mybir as mybir
import concourse.tile as tile
from concourse.bass_utils import run_bass_kernel_spmd

IN_F = 4096
OUT_F = 4096
NVALS = OUT_F + IN_F - 1  # 8191
B, S = 4, 2048
ROWS = B * S              # 8192
N_CORES = 8
M_PER_CORE = ROWS // N_CORES  # 1024

MT = 128
N_MC = M_PER_CORE // MT   # 8 row-tiles per core
N_KC = IN_F // 128        # 32 k-chunks of 128
LW = 512                  # Karatsuba leaf width
LKC = LW // 128           # 4 k-chunks per leaf
TBW = (LKC - 1) * 128 + LW  # 896: leaf table width
N_LEAF = 27

N_WARM = 28               # PE warm-up matmuls during startup DMA wait

BF16 = mybir.dt.bfloat16
F16 = mybir.dt.float16
F32 = mybir.dt.float32

# L2-node processing order (a1, a2); slice-only nodes first so the first
# matmuls need only the tail quarter of xt. Leaves within a node: a3 in
# (2, 3, 0): the two slice leaves (no combo dependency) run first, giving
# DVE ~1.7us of PE runway to finish the node's s3 combo before P0 needs it.
NODE_ORDER = [(2, 2), (2, 3), (2, 0), (3, 2), (3, 3), (3, 0),
              (0, 2), (0, 3), (0, 0)]
# last row-tile: group 0 first, group 3 finished half-by-half
LAST_ORDER = [(0, 2), (0, 3), (0, 0), (2, 2), (2, 3), (2, 0),
              (3, 2), (3, 0), (3, 3)]
LEAF_A3 = (2, 3, 0)
LEAF_ORDER = [(a1, a2, a3) for (a1, a2) in NODE_ORDER for a3 in LEAF_A3]
LEAF_IDX = {p: i for i, p in enumerate(LEAF_ORDER)}

_COMPILED = None


def _leaf_gens():
    """Leaf generators as {shift: coeff} over v(t) = vals[t mod 8191]."""
    gens = {}

    def sub(a, b):
        r = dict(a)
        for s, c in b.items():
            r[s] = r.get(s, 0) - c
            if r[s] == 0:
                del r[s]
        return r

    def rec(gen, w, path):
        if w == LW:
            gens[path] = gen
            return
        h = w // 2
        g_b = {s + h: c for s, c in gen.items()}
        g_c = {s - h: c for s, c in gen.items()}
        rec(gen, h, path + (0,))
        rec(sub(g_b, gen), h, path + (2,))
        rec(sub(g_c, gen), h, path + (3,))

    rec({0: 1}, IN_F, ())
    return gens


def _build_tables(diagonals):
    """[27, 128, 896] bf16 leaf tables; tbl[p, u] = g(p - u + 384)."""
    vals = np.concatenate(
        [diagonals[OUT_F - 1:], diagonals[: OUT_F - 1]]
    ).astype(np.float64)
    gens = _leaf_gens()
    t_idx = np.arange(-(LW - 1), LW)
    p = np.arange(128)[:, None]
    u = np.arange(TBW)[None, :]
    tbls = np.zeros((N_LEAF, 128, TBW), np.float64)
    for li, path in enumerate(LEAF_ORDER):
        g = np.zeros(2 * LW - 1)
        for s, c in gens[path].items():
            g += c * vals[np.mod(t_idx + s, NVALS)]
        tbls[li] = g[(p - u + 384) + (LW - 1)]
    # DRAM layout [p, leaf, u] so batched multi-leaf DMAs match the
    # SBUF [p, leaf, u] destination axis order
    return np.ascontiguousarray(
        tbls.astype(ml_dtypes.bfloat16).transpose(1, 0, 2)
    )


def _legalize_single_wait(nc):
    """This walrus build encodes at most one sync-wait per instruction;
    move extra waits onto carrier NoOps on the same engine."""
    for f in nc.m.functions:
        for blk in f.blocks:
            insts = blk.instructions
            new = []
            changed = False
            for inst in insts:
                si = inst.sync_info
                if si is not None and si.on_wait is not None and len(si.on_wait) > 1:
                    waits = list(si.on_wait)
                    for w in waits[:-1]:
                        nop = mybir.InstNoOp(name=f"I-waitsplit-{nc.next_id()}")
                        nop.engine = inst.engine
                        nop.sync_info = bass_rust.SyncInfo(on_wait=[w], on_update=[])
                        new.append(nop)
                    inst.sync_info = bass_rust.SyncInfo(
                        on_wait=[waits[-1]], on_update=si.on_update
                    )
                    changed = True
                new.append(inst)
            if changed:
                blk.instructions = new


def build_nc():
    nc = bass.Bass()
    # host layout: [mc][p][kc][m], kc ascending
    xt = nc.dram_tensor("xt", [N_MC, 128, N_KC, MT], BF16, kind="ExternalInput")
    tbl = nc.dram_tensor("tbl", [128, N_LEAF, TBW], BF16, kind="ExternalInput")
    bias_row = nc.dram_tensor("bias_row", [1, OUT_F], F16, kind="ExternalInput")
    y = nc.dram_tensor("y", [M_PER_CORE, OUT_F], F16, kind="ExternalOutput")

    with tile.TileContext(nc) as tc:
        with (
            tc.tile_pool(name="const", bufs=1) as cpool,
            tc.tile_pool(name="xp", bufs=3) as xpool,
            tc.tile_pool(name="cb", bufs=2) as cbpool,
            tc.tile_pool(name="l2", bufs=2) as l2pool,
            tc.tile_pool(name="l1", bufs=3) as l1pool,
            tc.tile_pool(name="ot", bufs=2) as opool,
            tc.tile_pool(name="sp", bufs=3) as spool,
            tc.tile_pool(name="pp", bufs=2, space="PSUM") as ppool,
            tc.tile_pool(name="wm", bufs=1) as wpool,
            tc.tile_pool(name="wp", bufs=1, space="PSUM") as wppool,
        ):
            tbl_sb = cpool.tile([128, N_LEAF, TBW], BF16)
            bias_sb = cpool.tile([128, OUT_F], F16)
            xt_first = xpool.tile([128, N_KC, MT], BF16, tag="xt")

            # PE warm-up on zeroed scratch so the HAM clock gate is at full
            # rate when the first real matmul issues.
            warm_sb = wpool.tile([128, 2 * MT], BF16)
            warm_ps = wppool.tile([MT, MT], F32)
            nc.vector.memset(warm_sb, 0)
            for _ in range(N_WARM):
                nc.tensor.matmul(
                    warm_ps, warm_sb[:, 0:MT], warm_sb[:, MT : 2 * MT],
                    start=True, stop=True, skip_group_check=True,
                )

            # Startup loads in first-need order across both HWDGE queues.
            # Later tables go as 4-table batched DMAs: per-table triggers
            # serialize on ring slots and starve the PE around t=20-27us.
            nc.sync.dma_start(out=xt_first[:, 24:32, :], in_=xt[0, :, 24:32, :])
            nc.scalar.dma_start(out=tbl_sb[:, 0, 384:TBW], in_=tbl[:, 0, 384:TBW])
            nc.sync.dma_start(out=tbl_sb[:, 1, :], in_=tbl[:, 1, :])
            nc.scalar.dma_start(out=tbl_sb[:, 0, 0:384], in_=tbl[:, 0, 0:384])
            nc.scalar.dma_start(out=tbl_sb[:, 2, :], in_=tbl[:, 2, :])
            nc.sync.dma_start(out=xt_first[:, 16:24, :], in_=xt[0, :, 16:24, :])
            nc.scalar.dma_start(out=tbl_sb[:, 3:7, :], in_=tbl[:, 3:7, :])
            nc.sync.dma_start(out=tbl_sb[:, 7:11, :], in_=tbl[:, 7:11, :])
            nc.scalar.dma_start(out=xt_first[:, 8:16, :], in_=xt[0, :, 8:16, :])
            nc.sync.dma_start(out=tbl_sb[:, 11:15, :], in_=tbl[:, 11:15, :])
            nc.scalar.dma_start(out=tbl_sb[:, 15:19, :], in_=tbl[:, 15:19, :])
            nc.sync.dma_start(out=xt_first[:, 0:8, :], in_=xt[0, :, 0:8, :])
            nc.scalar.dma_start(
                out=bias_sb,
                in_=bias_row[0:1, :].partition_broadcast(128).squeeze(1),
            )
            nc.sync.dma_start(out=tbl_sb[:, 19:23, :], in_=tbl[:, 19:23, :])
            nc.scalar.dma_start(out=tbl_sb[:, 23:27, :], in_=tbl[:, 23:27, :])

            def emit_combos(xt_sb):
                """Input combo tree for one row-tile (bf16, all DVE), in
                node-need order so mc0 can start on partially-loaded xt."""
                cb = {}
                u2 = xt_sb[:, 16:32, :]
                cb["s3_22"] = cbpool.tile([128, 4, MT], BF16, tag="s3_22", name="s3_22")
                nc.vector.tensor_add(cb["s3_22"], u2[:, 8:12, :], u2[:, 12:16, :])
                cb["s3_23"] = cbpool.tile([128, 4, MT], BF16, tag="s3_23", name="s3_23")
                nc.vector.tensor_add(cb["s3_23"], u2[:, 0:4, :], u2[:, 4:8, :])
                s2_2 = cbpool.tile([128, 8, MT], BF16, tag="s2_2", name="s2_2")
                nc.vector.tensor_add(s2_2, u2[:, 0:8, :], u2[:, 8:16, :])
                cb["s2_2"] = s2_2
                cb["s3_20"] = cbpool.tile([128, 4, MT], BF16, tag="s3_20", name="s3_20")
                nc.vector.tensor_add(cb["s3_20"], s2_2[:, 0:4, :], s2_2[:, 4:8, :])
                u3 = xt_sb[:, 0:16, :]
                cb["s3_32"] = cbpool.tile([128, 4, MT], BF16, tag="s3_32", name="s3_32")
                nc.vector.tensor_add(cb["s3_32"], u3[:, 8:12, :], u3[:, 12:16, :])
                cb["s3_33"] = cbpool.tile([128, 4, MT], BF16, tag="s3_33", name="s3_33")
                nc.vector.tensor_add(cb["s3_33"], u3[:, 0:4, :], u3[:, 4:8, :])
                s2_3 = cbpool.tile([128, 8, MT], BF16, tag="s2_3", name="s2_3")
                nc.vector.tensor_add(s2_3, u3[:, 0:8, :], u3[:, 8:16, :])
                cb["s2_3"] = s2_3
                cb["s3_30"] = cbpool.tile([128, 4, MT], BF16, tag="s3_30", name="s3_30")
                nc.vector.tensor_add(cb["s3_30"], s2_3[:, 0:4, :], s2_3[:, 4:8, :])
                s1 = cbpool.tile([128, 16, MT], BF16, tag="s1", name="s1")
                nc.vector.tensor_add(s1, xt_sb[:, 0:16, :], xt_sb[:, 16:32, :])
                cb["s1"] = s1
                cb["s3_02"] = cbpool.tile([128, 4, MT], BF16, tag="s3_02", name="s3_02")
                nc.vector.tensor_add(cb["s3_02"], s1[:, 8:12, :], s1[:, 12:16, :])
                cb["s3_03"] = cbpool.tile([128, 4, MT], BF16, tag="s3_03", name="s3_03")
                nc.vector.tensor_add(cb["s3_03"], s1[:, 0:4, :], s1[:, 4:8, :])
                s2_0 = cbpool.tile([128, 8, MT], BF16, tag="s2_0", name="s2_0")
                nc.vector.tensor_add(s2_0, s1[:, 0:8, :], s1[:, 8:16, :])
                cb["s2_0"] = s2_0
                cb["s3_00"] = cbpool.tile([128, 4, MT], BF16, tag="s3_00", name="s3_00")
                nc.vector.tensor_add(cb["s3_00"], s2_0[:, 0:4, :], s2_0[:, 4:8, :])
                return cb

            def process_node(a1, a2, xt_sb, combos):
                """Matmuls + drains + nl/nr adds for one L2 node; returns
                the node's f16 [128, 1024] (nl|nr) output tile."""
                if a2 == 0:
                    v_v = combos[f"s2_{a1}"]
                else:
                    u_v = {0: combos["s1"], 2: xt_sb[:, 16:32, :],
                           3: xt_sb[:, 0:16, :]}[a1]
                    v_v = u_v[:, 8:16, :] if a2 == 2 else u_v[:, 0:8, :]
                s3 = combos[f"s3_{a1}{a2}"]
                lhss = {2: v_v[:, 4:8, :], 3: v_v[:, 0:4, :], 0: s3}

                sbs = {}
                for a3 in LEAF_A3:
                    li = LEAF_IDX[(a1, a2, a3)]
                    acc = ppool.tile(
                        [128, LW], F32, tag=f"pp{a3}", name=f"pp{a3}",
                        bufs=3 if a3 == 0 else 2,
                    )
                    lhs = lhss[a3]
                    for kk in range(LKC):
                        c = (LKC - 1 - kk) * 128
                        nc.tensor.matmul(
                            acc, lhs[:, kk, :], tbl_sb[:, li, c : c + LW],
                            start=(kk == 0), stop=(kk == LKC - 1),
                        )
                    psb = spool.tile([128, LW], F16, tag=f"p{a3}sb", name=f"p{a3}sb")
                    nc.scalar.copy(psb, acc)
                    sbs[a3] = psb

                l2t = l2pool.tile([128, 2 * LW], F16, tag=f"c{a2}", name=f"c{a2}")
                nc.vector.tensor_add(l2t[:, 0:LW], sbs[2], sbs[0])
                nc.vector.tensor_add(l2t[:, LW : 2 * LW], sbs[3], sbs[0])
                return l2t

            def emit_l1(a1, l2outs):
                l1t = l1pool.tile([128, 4 * LW], F16, tag=f"u{a1}", name=f"u{a1}")
                nc.vector.tensor_add(l1t[:, 0 : 2 * LW], l2outs[0], l2outs[2])
                nc.vector.tensor_add(l1t[:, 2 * LW : 4 * LW], l2outs[0], l2outs[3])
                return l1t

            xt_tiles = {0: xt_first}
            combos = emit_combos(xt_first)
            h = OUT_F // 2
            q = OUT_F // 4
            pend_roots = []

            def emit_root(m0r, l1o):
                tl = opool.tile([128, h], F16, tag="t", name="tl")
                nc.vector.tensor_add(tl, l1o[0], l1o[2])
                outl = opool.tile([128, h], F16, tag="o", name="outl")
                nc.vector.tensor_add(outl, tl, bias_sb[:, 0:h])
                nc.scalar.dma_start(out=y[m0r : m0r + MT, 0:h], in_=outl)
                tr = opool.tile([128, h], F16, tag="t", name="tr")
                nc.vector.tensor_add(tr, l1o[0], l1o[3])
                outr = opool.tile([128, h], F16, tag="o", name="outr")
                nc.vector.tensor_add(outr, tr, bias_sb[:, h:OUT_F])
                nc.scalar.dma_start(out=y[m0r : m0r + MT, h:OUT_F], in_=outr)
            for mc in range(N_MC):
                m0 = mc * MT
                xt_sb = xt_tiles.pop(mc)
                last = mc == N_MC - 1
                # prefetch two ahead so mc+1's combos never wait on DMA
                nxts = (1, 2) if mc == 0 else (mc + 2,)
                for nxt in nxts:
                    if nxt < N_MC:
                        xt_pref = xpool.tile(
                            [128, N_KC, MT], BF16, tag="xt", name="xt_pref"
                        )
                        nc.sync.dma_start(out=xt_pref, in_=xt[nxt, :, :, :])
                        xt_tiles[nxt] = xt_pref

                if not last:
                    l2outs = {}
                    l1outs = {}
                    deferred = None
                    nxt_combos = None
                    for ni, (a1, a2) in enumerate(NODE_ORDER):
                        l2outs[a2] = process_node(a1, a2, xt_sb, combos)
                        if ni == 5 and mc >= 1:
                            # next row-tile's combos go on DVE ahead of
                            # group 0's adds and the tail so the PE rolls
                            # over the boundary with ~7us of slack (at mc0
                            # xt1 is still in flight, so emit at the end)
                            nxt_combos = emit_combos(xt_tiles[mc + 1])
                        if ni % 3 != 2:
                            continue
                        if a1 == 0:
                            deferred = dict(l2outs)
                        else:
                            l1outs[a1] = emit_l1(a1, l2outs)

                    if nxt_combos is None:
                        nxt_combos = emit_combos(xt_tiles[mc + 1])
                    combos = nxt_combos
                    l1outs[0] = emit_l1(0, deferred)

                    # defer this tile's root adds two tiles: DVE enters the
                    # first tiles ~7us behind (mc0 combo bootstrap) and the
                    # root work is the only slack-tolerant piece
                    pend_roots.append((m0, dict(l1outs)))
                    if len(pend_roots) > 2:
                        emit_root(*pend_roots.pop(0))
                    continue

                # ---- last row-tile: minimize post-matmul tail ----
                for pr in pend_roots:
                    emit_root(*pr)
                pend_roots = []
                l2outs = {}
                u0 = None
                for a1, a2 in LAST_ORDER[:6]:
                    l2outs[a2] = process_node(a1, a2, xt_sb, combos)
                    if a2 == 0 and a1 == 0:
                        u0 = emit_l1(0, l2outs)
                    elif a2 == 0 and a1 == 2:
                        u2 = emit_l1(2, l2outs)
                        tl = opool.tile([128, h], F16, tag="t", name="tl")
                        nc.vector.tensor_add(tl, u0, u2)
                        outl = opool.tile([128, h], F16, tag="o", name="outl")
                        nc.vector.tensor_add(outl, tl, bias_sb[:, 0:h])
                        nc.scalar.dma_start(out=y[m0 : m0 + MT, 0:h], in_=outl)
                c2 = process_node(3, 2, xt_sb, combos)
                c0 = process_node(3, 0, xt_sb, combos)
                # right-left 1024 finishes before the last node's matmuls
                ml = l1pool.tile([128, 2 * LW], F16, tag="u3", name="ml3")
                nc.vector.tensor_add(ml, c0, c2)
                trm = opool.tile([128, q], F16, tag="t", name="trm")
                nc.vector.tensor_add(trm, u0[:, 0 : 2 * LW], ml)
                outrm = opool.tile([128, q], F16, tag="o", name="outrm")
                nc.vector.tensor_add(outrm, trm, bias_sb[:, h : h + q])
                nc.scalar.dma_start(out=y[m0 : m0 + MT, h : h + q], in_=outrm)
                c3 = process_node(3, 3, xt_sb, combos)
                mr = l1pool.tile([128, 2 * LW], F16, tag="u2", name="mr3")
                nc.vector.tensor_add(mr, c0, c3)
                trr = opool.tile([128, q], F16, tag="t", name="trr")
                nc.vector.tensor_add(trr, u0[:, 2 * LW : 4 * LW], mr)
                outrr = opool.tile([128, q], F16, tag="o", name="outrr")
                nc.vector.tensor_add(outrr, trr, bias_sb[:, h + q : OUT_F])
                e = h + q + LW
                nc.sync.dma_start(out=y[m0 : m0 + MT, h + q : e], in_=outrr[:, 0:LW])
                nc.scalar.dma_start(out=y[m0 : m0 + MT, e:OUT_F], in_=outrr[:, LW:q])

    _legalize_single_wait(nc)
    return nc


def make_in_maps(x, diagonals, bias):
    x = np.asarray(x, dtype=np.float32)
    diagonals = np.asarray(diagonals, dtype=np.float32)
    bias = np.asarray(bias, dtype=np.float32)

    tbls = _build_tables(diagonals.astype(np.float64))
    bias_row = np.ascontiguousarray(bias.astype(np.float16).reshape(1, OUT_F))
    x16 = x.reshape(ROWS, IN_F).astype(ml_dtypes.bfloat16)

    in_maps = []
    for c in range(N_CORES):
        xc = x16[c * M_PER_CORE : (c + 1) * M_PER_CORE]  # [1024, 4096]
        x4 = xc.reshape(N_MC, MT, N_KC, 128)  # [mc, m, kc, p]
        x4 = np.ascontiguousarray(x4.transpose(0, 3, 2, 1))  # [mc, p, kc, m]
        in_maps.append({"xt": x4, "tbl": tbls, "bias_row": bias_row})
    return in_maps


def kernel(x, diagonals, bias):
    global _COMPILED
    if _COMPILED is None:
        _COMPILED = build_nc()
    nc = _COMPILED

    in_maps = make_in_maps(x, diagonals, bias)
    res = run_bass_kernel_spmd(nc, in_maps, core_ids=list(range(N_CORES)))
    y = np.concatenate(
        [np.asarray(res.results[c]["y"]) for c in range(N_CORES)], axis=0
    )
    return y.astype(np.float32).reshape(B, S, OUT_F)


# revision 26
# speedup vs baseline: 1.0134x; 1.0134x over previous
"""DiagonalLinear (Toeplitz linear) Trainium2 kernel — Karatsuba v14.

y[b,s,o] = sum_i x[b,s,i] * W[o,i] + bias[o],  W[o,i] = vals[(i-o) mod 8191]
x: [4, 2048, 4096] f32 -> bf16 operands, f32 PSUM, f16 output (cast on host).

Data-parallel over 8 cores (1024 rows each). Within a core, the 4096x4096
Toeplitz matmul is decomposed with 3 levels of Karatsuba on the 2x2 block
structure W = [[A,B],[C,A]] (diagonal blocks of a Toeplitz matrix repeat):
  y_left  = A x0 + B x1 = P0 + P2,   P0 = A(x0+x1), P2 = (B-A) x1
  y_right = C x0 + A x1 = P0 + P3,   P3 = (C-A) x0
Recursing 3x gives 27 leaf products of [512x512] Toeplitz blocks = 108
N=512 matmuls per 128-row tile instead of 256 (42% of the MACs). Each
leaf block is a free-dim slice of a [128 x 896] periodic table built
host-side from +/- combinations of shifted `vals`.

Engine split (measured rates): PE runs the 864 matmuls gap-free at the
215.8ns N=512 issue floor; Scalar drains all 27 leaf PSUMs per row-tile
to f16 SBUF (~690ns each); DVE does every add in 16-bit SBUF (2x mode,
~425ns per 512-col add) — input combo tree, nl/nr, L1, root+bias.
GpSimd is unused for compute: it shares SBUF ports with DVE and running
them concurrently stalls DVE ~6x. The next row-tile's combos are emitted
on DVE before the current tile's tail adds, and each tile's root adds
are deferred two tiles (DVE enters the first tiles ~7us behind from the
mc0 combo bootstrap), so the PE stream never breaks at a row-tile
boundary. Startup orders table/xt DMAs by first-need on both HWDGE
queues with later tables as 4-table batched transfers (startup is
supply-bound at ~0.43MB/us from t~8us; reordering cannot beat it). The
last row-tile runs group 0 first and finishes group 3 in column halves
to shorten the tail. HW: ~224us on 8 cores (baseline 464us), rel err
~5.8e-3 (gate 2e-2).
"""

import numpy as np
import ml_dtypes

import bass_rust
import concourse.bass as bass
import concourse.mybir as mybir
import concourse.tile as tile
from concourse.bass_utils import run_bass_kernel_spmd

IN_F = 4096
OUT_F = 4096
NVALS = OUT_F + IN_F - 1  # 8191
B, S = 4, 2048
ROWS = B * S              # 8192
N_CORES = 8
M_PER_CORE = ROWS // N_CORES  # 1024

MT = 128
N_MC = M_PER_CORE // MT   # 8 row-tiles per core
N_KC = IN_F // 128        # 32 k-chunks of 128
LW = 512                  # Karatsuba leaf width
LKC = LW // 128           # 4 k-chunks per leaf
TBW = (LKC - 1) * 128 + LW  # 896: leaf table width
N_LEAF = 27

N_WARM = 28               # PE warm-up matmuls during startup DMA wait

BF16 = mybir.dt.bfloat16
F16 = mybir.dt.float16
F32 = mybir.dt.float32

# L2-node processing order (a1, a2); slice-only nodes first so the first
# matmuls need only the tail quarter of xt. Leaves within a node: a3 in
# (2, 3, 0): the two slice leaves (no combo dependency) run first, giving
# DVE ~1.7us of PE runway to finish the node's s3 combo before P0 needs it.
NODE_ORDER = [(2, 2), (2, 3), (2, 0), (3, 2), (3, 3), (3, 0),
              (0, 2), (0, 3), (0, 0)]
# last row-tile: group 0 first, group 3 finished half-by-half
LAST_ORDER = [(0, 2), (0, 3), (0, 0), (2, 2), (2, 3), (2, 0),
              (3, 2), (3, 0), (3, 3)]
LEAF_A3 = (2, 3, 0)
LEAF_ORDER = [(a1, a2, a3) for (a1, a2) in NODE_ORDER for a3 in LEAF_A3]
LEAF_IDX = {p: i for i, p in enumerate(LEAF_ORDER)}

_COMPILED = None


def _leaf_gens():
    """Leaf generators as {shift: coeff} over v(t) = vals[t mod 8191]."""
    gens = {}

    def sub(a, b):
        r = dict(a)
        for s, c in b.items():
            r[s] = r.get(s, 0) - c
            if r[s] == 0:
                del r[s]
        return r

    def rec(gen, w, path):
        if w == LW:
            gens[path] = gen
            return
        h = w // 2
        g_b = {s + h: c for s, c in gen.items()}
        g_c = {s - h: c for s, c in gen.items()}
        rec(gen, h, path + (0,))
        rec(sub(g_b, gen), h, path + (2,))
        rec(sub(g_c, gen), h, path + (3,))

    rec({0: 1}, IN_F, ())
    return gens


def _build_tables(diagonals):
    """[27, 128, 896] bf16 leaf tables; tbl[p, u] = g(p - u + 384)."""
    vals = np.concatenate(
        [diagonals[OUT_F - 1:], diagonals[: OUT_F - 1]]
    ).astype(np.float64)
    gens = _leaf_gens()
    t_idx = np.arange(-(LW - 1), LW)
    p = np.arange(128)[:, None]
    u = np.arange(TBW)[None, :]
    tbls = np.zeros((N_LEAF, 128, TBW), np.float64)
    for li, path in enumerate(LEAF_ORDER):
        g = np.zeros(2 * LW - 1)
        for s, c in gens[path].items():
            g += c * vals[np.mod(t_idx + s, NVALS)]
        tbls[li] = g[(p - u + 384) + (LW - 1)]
    # DRAM layout [p, leaf, u] so batched multi-leaf DMAs match the
    # SBUF [p, leaf, u] destination axis order
    return np.ascontiguousarray(
        tbls.astype(ml_dtypes.bfloat16).transpose(1, 0, 2)
    )


def _legalize_single_wait(nc):
    """This walrus build encodes at most one sync-wait per instruction;
    move extra waits onto carrier NoOps on the same engine."""
    for f in nc.m.functions:
        for blk in f.blocks:
            insts = blk.instructions
            new = []
            changed = False
            for inst in insts:
                si = inst.sync_info
                if si is not None and si.on_wait is not None and len(si.on_wait) > 1:
                    waits = list(si.on_wait)
                    for w in waits[:-1]:
                        nop = mybir.InstNoOp(name=f"I-waitsplit-{nc.next_id()}")
                        nop.engine = inst.engine
                        nop.sync_info = bass_rust.SyncInfo(on_wait=[w], on_update=[])
                        new.append(nop)
                    inst.sync_info = bass_rust.SyncInfo(
                        on_wait=[waits[-1]], on_update=si.on_update
                    )
                    changed = True
                new.append(inst)
            if changed:
                blk.instructions = new


def build_nc():
    nc = bass.Bass()
    # host layout: [mc][p][kc][m], kc ascending
    xt = nc.dram_tensor("xt", [N_MC, 128, N_KC, MT], BF16, kind="ExternalInput")
    tbl = nc.dram_tensor("tbl", [128, N_LEAF, TBW], BF16, kind="ExternalInput")
    bias_row = nc.dram_tensor("bias_row", [1, OUT_F], F16, kind="ExternalInput")
    y = nc.dram_tensor("y", [M_PER_CORE, OUT_F], F16, kind="ExternalOutput")

    with tile.TileContext(nc) as tc:
        with (
            tc.tile_pool(name="const", bufs=1) as cpool,
            tc.tile_pool(name="xp", bufs=3) as xpool,
            tc.tile_pool(name="cb", bufs=2) as cbpool,
            tc.tile_pool(name="l2", bufs=2) as l2pool,
            tc.tile_pool(name="l1", bufs=3) as l1pool,
            tc.tile_pool(name="ot", bufs=2) as opool,
            tc.tile_pool(name="sp", bufs=3) as spool,
            tc.tile_pool(name="pp", bufs=2, space="PSUM") as ppool,
            tc.tile_pool(name="wm", bufs=1) as wpool,
            tc.tile_pool(name="wp", bufs=1, space="PSUM") as wppool,
        ):
            tbl_sb = cpool.tile([128, N_LEAF, TBW], BF16)
            bias_sb = cpool.tile([128, OUT_F], F16)
            xt_first = xpool.tile([128, N_KC, MT], BF16, tag="xt")

            # PE warm-up on zeroed scratch so the HAM clock gate is at full
            # rate when the first real matmul issues.
            warm_sb = wpool.tile([128, 2 * MT], BF16)
            warm_ps = wppool.tile([MT, MT], F32)
            nc.vector.memset(warm_sb, 0)
            for _ in range(N_WARM):
                nc.tensor.matmul(
                    warm_ps, warm_sb[:, 0:MT], warm_sb[:, MT : 2 * MT],
                    start=True, stop=True, skip_group_check=True,
                )

            # Startup loads in first-need order across both HWDGE queues.
            # Later tables go as 4-table batched DMAs: per-table triggers
            # serialize on ring slots and starve the PE around t=20-27us.
            nc.sync.dma_start(out=xt_first[:, 24:32, :], in_=xt[0, :, 24:32, :])
            nc.scalar.dma_start(out=tbl_sb[:, 0, 384:TBW], in_=tbl[:, 0, 384:TBW])
            nc.sync.dma_start(out=tbl_sb[:, 1, :], in_=tbl[:, 1, :])
            nc.scalar.dma_start(out=tbl_sb[:, 0, 0:384], in_=tbl[:, 0, 0:384])
            nc.scalar.dma_start(out=tbl_sb[:, 2, :], in_=tbl[:, 2, :])
            nc.sync.dma_start(out=xt_first[:, 16:24, :], in_=xt[0, :, 16:24, :])
            nc.scalar.dma_start(out=tbl_sb[:, 3:7, :], in_=tbl[:, 3:7, :])
            nc.sync.dma_start(out=tbl_sb[:, 7:11, :], in_=tbl[:, 7:11, :])
            nc.scalar.dma_start(out=xt_first[:, 8:16, :], in_=xt[0, :, 8:16, :])
            nc.sync.dma_start(out=tbl_sb[:, 11:15, :], in_=tbl[:, 11:15, :])
            nc.scalar.dma_start(out=tbl_sb[:, 15:19, :], in_=tbl[:, 15:19, :])
            nc.sync.dma_start(out=xt_first[:, 0:8, :], in_=xt[0, :, 0:8, :])
            nc.scalar.dma_start(
                out=bias_sb,
                in_=bias_row[0:1, :].partition_broadcast(128).squeeze(1),
            )
            nc.sync.dma_start(out=tbl_sb[:, 19:23, :], in_=tbl[:, 19:23, :])
            nc.scalar.dma_start(out=tbl_sb[:, 23:27, :], in_=tbl[:, 23:27, :])

            def emit_combos(xt_sb):
                """Input combo tree for one row-tile (bf16, all DVE), in
                node-need order so mc0 can start on partially-loaded xt."""
                cb = {}
                u2 = xt_sb[:, 16:32, :]
                cb["s3_22"] = cbpool.tile([128, 4, MT], BF16, tag="s3_22", name="s3_22")
                nc.vector.tensor_add(cb["s3_22"], u2[:, 8:12, :], u2[:, 12:16, :])
                cb["s3_23"] = cbpool.tile([128, 4, MT], BF16, tag="s3_23", name="s3_23")
                nc.vector.tensor_add(cb["s3_23"], u2[:, 0:4, :], u2[:, 4:8, :])
                s2_2 = cbpool.tile([128, 8, MT], BF16, tag="s2_2", name="s2_2")
                nc.vector.tensor_add(s2_2, u2[:, 0:8, :], u2[:, 8:16, :])
                cb["s2_2"] = s2_2
                cb["s3_20"] = cbpool.tile([128, 4, MT], BF16, tag="s3_20", name="s3_20")
                nc.vector.tensor_add(cb["s3_20"], s2_2[:, 0:4, :], s2_2[:, 4:8, :])
                u3 = xt_sb[:, 0:16, :]
                cb["s3_32"] = cbpool.tile([128, 4, MT], BF16, tag="s3_32", name="s3_32")
                nc.vector.tensor_add(cb["s3_32"], u3[:, 8:12, :], u3[:, 12:16, :])
                cb["s3_33"] = cbpool.tile([128, 4, MT], BF16, tag="s3_33", name="s3_33")
                nc.vector.tensor_add(cb["s3_33"], u3[:, 0:4, :], u3[:, 4:8, :])
                s2_3 = cbpool.tile([128, 8, MT], BF16, tag="s2_3", name="s2_3")
                nc.vector.tensor_add(s2_3, u3[:, 0:8, :], u3[:, 8:16, :])
                cb["s2_3"] = s2_3
                cb["s3_30"] = cbpool.tile([128, 4, MT], BF16, tag="s3_30", name="s3_30")
                nc.vector.tensor_add(cb["s3_30"], s2_3[:, 0:4, :], s2_3[:, 4:8, :])
                s1 = cbpool.tile([128, 16, MT], BF16, tag="s1", name="s1")
                nc.vector.tensor_add(s1, xt_sb[:, 0:16, :], xt_sb[:, 16:32, :])
                cb["s1"] = s1
                cb["s3_02"] = cbpool.tile([128, 4, MT], BF16, tag="s3_02", name="s3_02")
                nc.vector.tensor_add(cb["s3_02"], s1[:, 8:12, :], s1[:, 12:16, :])
                cb["s3_03"] = cbpool.tile([128, 4, MT], BF16, tag="s3_03", name="s3_03")
                nc.vector.tensor_add(cb["s3_03"], s1[:, 0:4, :], s1[:, 4:8, :])
                s2_0 = cbpool.tile([128, 8, MT], BF16, tag="s2_0", name="s2_0")
                nc.vector.tensor_add(s2_0, s1[:, 0:8, :], s1[:, 8:16, :])
                cb["s2_0"] = s2_0
                cb["s3_00"] = cbpool.tile([128, 4, MT], BF16, tag="s3_00", name="s3_00")
                nc.vector.tensor_add(cb["s3_00"], s2_0[:, 0:4, :], s2_0[:, 4:8, :])
                return cb

            def process_node(a1, a2, xt_sb, combos):
                """Matmuls + drains + nl/nr adds for one L2 node; returns
                the node's f16 [128, 1024] (nl|nr) output tile."""
                if a2 == 0:
                    v_v = combos[f"s2_{a1}"]
                else:
                    u_v = {0: combos["s1"], 2: xt_sb[:, 16:32, :],
                           3: xt_sb[:, 0:16, :]}[a1]
                    v_v = u_v[:, 8:16, :] if a2 == 2 else u_v[:, 0:8, :]
                s3 = combos[f"s3_{a1}{a2}"]
                lhss = {2: v_v[:, 4:8, :], 3: v_v[:, 0:4, :], 0: s3}

                sbs = {}
                for a3 in LEAF_A3:
                    li = LEAF_IDX[(a1, a2, a3)]
                    acc = ppool.tile(
                        [128, LW], F32, tag=f"pp{a3}", name=f"pp{a3}",
                        bufs=3 if a3 == 0 else 2,
                    )
                    lhs = lhss[a3]
                    for kk in range(LKC):
                        c = (LKC - 1 - kk) * 128
                        nc.tensor.matmul(
                            acc, lhs[:, kk, :], tbl_sb[:, li, c : c + LW],
                            start=(kk == 0), stop=(kk == LKC - 1),
                        )
                    psb = spool.tile([128, LW], F16, tag=f"p{a3}sb", name=f"p{a3}sb")
                    nc.scalar.copy(psb, acc)
                    sbs[a3] = psb

                l2t = l2pool.tile([128, 2 * LW], F16, tag=f"c{a2}", name=f"c{a2}")
                nc.vector.tensor_add(l2t[:, 0:LW], sbs[2], sbs[0])
                nc.vector.tensor_add(l2t[:, LW : 2 * LW], sbs[3], sbs[0])
                return l2t

            def emit_l1(a1, l2outs):
                l1t = l1pool.tile([128, 4 * LW], F16, tag=f"u{a1}", name=f"u{a1}")
                nc.vector.tensor_add(l1t[:, 0 : 2 * LW], l2outs[0], l2outs[2])
                nc.vector.tensor_add(l1t[:, 2 * LW : 4 * LW], l2outs[0], l2outs[3])
                return l1t

            xt_tiles = {0: xt_first}
            combos = emit_combos(xt_first)
            h = OUT_F // 2
            q = OUT_F // 4
            pend_roots = []

            def emit_root(m0r, l1o):
                tl = opool.tile([128, h], F16, tag="t", name="tl")
                nc.vector.tensor_add(tl, l1o[0], l1o[2])
                outl = opool.tile([128, h], F16, tag="o", name="outl")
                nc.vector.tensor_add(outl, tl, bias_sb[:, 0:h])
                nc.scalar.dma_start(out=y[m0r : m0r + MT, 0:h], in_=outl)
                tr = opool.tile([128, h], F16, tag="t", name="tr")
                nc.vector.tensor_add(tr, l1o[0], l1o[3])
                outr = opool.tile([128, h], F16, tag="o", name="outr")
                nc.vector.tensor_add(outr, tr, bias_sb[:, h:OUT_F])
                nc.scalar.dma_start(out=y[m0r : m0r + MT, h:OUT_F], in_=outr)
            for mc in range(N_MC):
                m0 = mc * MT
                xt_sb = xt_tiles.pop(mc)
                last = mc == N_MC - 1
                # prefetch two ahead so mc+1's combos never wait on DMA
                nxts = (1, 2) if mc == 0 else (mc + 2,)
                for nxt in nxts:
                    if nxt < N_MC:
                        xt_pref = xpool.tile(
                            [128, N_KC, MT], BF16, tag="xt", name="xt_pref"
                        )
                        nc.sync.dma_start(out=xt_pref, in_=xt[nxt, :, :, :])
                        xt_tiles[nxt] = xt_pref

                if not last:
                    l2outs = {}
                    l1outs = {}
                    deferred = None
                    nxt_combos = None
                    for ni, (a1, a2) in enumerate(NODE_ORDER):
                        l2outs[a2] = process_node(a1, a2, xt_sb, combos)
                        if ni == 5 and mc >= 1:
                            # next row-tile's combos go on DVE ahead of
                            # group 0's adds and the tail so the PE rolls
                            # over the boundary with ~7us of slack (at mc0
                            # xt1 is still in flight, so emit at the end)
                            nxt_combos = emit_combos(xt_tiles[mc + 1])
                        if ni % 3 != 2:
                            continue
                        if a1 == 0:
                            deferred = dict(l2outs)
                        else:
                            l1outs[a1] = emit_l1(a1, l2outs)

                    if nxt_combos is None:
                        nxt_combos = emit_combos(xt_tiles[mc + 1])
                    combos = nxt_combos
                    l1outs[0] = emit_l1(0, deferred)

                    # defer this tile's root adds two tiles: DVE enters the
                    # first tiles ~7us behind (mc0 combo bootstrap) and the
                    # root work is the only slack-tolerant piece
                    pend_roots.append((m0, dict(l1outs)))
                    if len(pend_roots) > 2:
                        emit_root(*pend_roots.pop(0))
                    continue

                # ---- last row-tile: minimize post-matmul tail ----
                for pr in pend_roots:
                    emit_root(*pr)
                pend_roots = []
                l2outs = {}
                u0 = None
                for a1, a2 in LAST_ORDER[:6]:
                    l2outs[a2] = process_node(a1, a2, xt_sb, combos)
                    if a2 == 0 and a1 == 0:
                        u0 = emit_l1(0, l2outs)
                    elif a2 == 0 and a1 == 2:
                        u2 = emit_l1(2, l2outs)
                        tl = opool.tile([128, h], F16, tag="t", name="tl")
                        nc.vector.tensor_add(tl, u0, u2)
                        outl = opool.tile([128, h], F16, tag="o", name="outl")
                        nc.vector.tensor_add(outl, tl, bias_sb[:, 0:h])
                        nc.scalar.dma_start(out=y[m0 : m0 + MT, 0:h], in_=outl)
                c2 = process_node(3, 2, xt_sb, combos)
                c0 = process_node(3, 0, xt_sb, combos)
                # right-left 1024 finishes before the last node's matmuls
                ml = l1pool.tile([128, 2 * LW], F16, tag="u3", name="ml3")
                nc.vector.tensor_add(ml, c0, c2)
                trm = opool.tile([128, q], F16, tag="t", name="trm")
                nc.vector.tensor_add(trm, u0[:, 0 : 2 * LW], ml)
                outrm = opool.tile([128, q], F16, tag="o", name="outrm")
                nc.vector.tensor_add(outrm, trm, bias_sb[:, h : h + q])
                nc.scalar.dma_start(out=y[m0 : m0 + MT, h : h + q], in_=outrm)
                c3 = process_node(3, 3, xt_sb, combos)
                mr = l1pool.tile([128, 2 * LW], F16, tag="u2", name="mr3")
                nc.vector.tensor_add(mr, c0, c3)
                trr = opool.tile([128, q], F16, tag="t", name="trr")
                nc.vector.tensor_add(trr, u0[:, 2 * LW : 4 * LW], mr)
                outrr = opool.tile([128, q], F16, tag="o", name="outrr")
                nc.vector.tensor_add(outrr, trr, bias_sb[:, h + q : OUT_F])
                e = h + q + LW
                nc.sync.dma_start(out=y[m0 : m0 + MT, h + q : e], in_=outrr[:, 0:LW])
                nc.scalar.dma_start(out=y[m0 : m0 + MT, e:OUT_F], in_=outrr[:, LW:q])

    _legalize_single_wait(nc)
    return nc


def make_in_maps(x, diagonals, bias):
    x = np.asarray(x, dtype=np.float32)
    diagonals = np.asarray(diagonals, dtype=np.float32)
    bias = np.asarray(bias, dtype=np.float32)

    tbls = _build_tables(diagonals.astype(np.float64))
    bias_row = np.ascontiguousarray(bias.astype(np.float16).reshape(1, OUT_F))
    x16 = x.reshape(ROWS, IN_F).astype(ml_dtypes.bfloat16)

    in_maps = []
    for c in range(N_CORES):
        xc = x16[c * M_PER_CORE : (c + 1) * M_PER_CORE]  # [1024, 4096]
        x4 = xc.reshape(N_MC, MT, N_KC, 128)  # [mc, m, kc, p]
        x4 = np.ascontiguousarray(x4.transpose(0, 3, 2, 1))  # [mc, p, kc, m]
        in_maps.append({"xt": x4, "tbl": tbls, "bias_row": bias_row})
    return in_maps


def kernel(x, diagonals, bias):
    global _COMPILED
    if _COMPILED is None:
        _COMPILED = build_nc()
    nc = _COMPILED

    in_maps = make_in_maps(x, diagonals, bias)
    res = run_bass_kernel_spmd(nc, in_maps, core_ids=list(range(N_CORES)))
    y = np.concatenate(
        [np.asarray(res.results[c]["y"]) for c in range(N_CORES)], axis=0
    )
    return y.astype(np.float32).reshape(B, S, OUT_F)
